# revision 12
# baseline (speedup 1.0000x reference)
"""Causal attention block (q/k/v proj + causal softmax + concat) on 8 trn2 cores.

Sharding: batch n -> core pair (2n, 2n+1); within a batch the 16 query
chunks of 256 rows are split by parity (core p owns chunks 2s+p, s=0..7).
The host hands each core a row-permuted x (own chunks first, then the
other core's chunks) plus its fp16 transpose, so all 8 cores run one
uniform SPMD program; causal-mask differences are pure input data
(multiplicative 0/1 mask tiles). Attention runs in the transposed
orientation (logits^T tiles [tk=128, tq=512]) over PAIRS of query slots,
with fp16 matmul operands (fp32 PSUM accumulation), and the softmax
denominator is fused into the read matmul via a ones-column appended
to v. Groups whose first-slot half is dead by causality are computed at
half width.
"""

from contextlib import ExitStack

import numpy as np

import concourse.bacc as bacc
import concourse.mybir as mybir
import concourse.tile as tile
from concourse.bass_utils import run_bass_kernel_spmd

F32 = mybir.dt.float32
F16 = mybir.dt.float16
ADD = mybir.AluOpType.add
EXP = mybir.ActivationFunctionType.Exp

N, T, C, KD, VD = 4, 4096, 512, 64, 64
CH = 256          # query chunk rows
SLOTS = 8         # own 256-chunks per core
TQ = SLOTS * CH   # 2048 own query rows per core
SCALE = 1.0 / 8.0

_NC_CACHE = None
_LAST_IN_MAPS = None

# DMA / projection processing order: own cols (h 0,1) and other cols (h 2,3)
# interleaved so early attention pairs have both parts available.
H_ORDER = [0, 2, 1, 3]
K_ORDER = [0, 1, 4, 5, 2, 3, 6, 7]


def _build():
    nc = bacc.Bacc("TRN2", target_bir_lowering=False, debug=False)

    xT_d = nc.dram_tensor("xT", [C, T], F16, kind="ExternalInput").ap()
    xq_d = nc.dram_tensor("xq", [TQ, C], F32, kind="ExternalInput").ap()
    wq_d = nc.dram_tensor("wq", [4, 128, KD], F16, kind="ExternalInput").ap()
    wk_d = nc.dram_tensor("wk", [4, 128, KD], F16, kind="ExternalInput").ap()
    wv_d = nc.dram_tensor("wv", [4, 128, VD], F16, kind="ExternalInput").ap()
    bq_d = nc.dram_tensor("bq", [128, 1], F32, kind="ExternalInput").ap()
    bk_d = nc.dram_tensor("bk", [128, 1], F32, kind="ExternalInput").ap()
    bv_d = nc.dram_tensor("bv", [VD, 1], F32, kind="ExternalInput").ap()
    masks_d = nc.dram_tensor("masks", [128, 4, CH], F32, kind="ExternalInput").ap()
    ident_d = nc.dram_tensor("ident", [128, 128], F32, kind="ExternalInput").ap()
    ident16_d = nc.dram_tensor("ident16", [128, 128], F16, kind="ExternalInput").ap()
    out_d = nc.dram_tensor("out", [TQ, C + VD], F32, kind="ExternalOutput").ap()

    with tile.TileContext(nc) as tc, ExitStack() as ctx:
        const = ctx.enter_context(tc.tile_pool(name="const", bufs=1))
        data = ctx.enter_context(tc.tile_pool(name="data", bufs=1))

        ident_sb = const.tile([128, 128], F32)
        nc.sync.dma_start(out=ident_sb, in_=ident_d)
        ident16_sb = const.tile([128, 128], F16)
        nc.sync.dma_start(out=ident16_sb, in_=ident16_d)
        masks_sb = const.tile([128, 4, CH], F32)
        nc.sync.dma_start(out=masks_sb, in_=masks_d)
        bqq_sb = const.tile([128, 1], F32)
        nc.sync.dma_start(out=bqq_sb, in_=bq_d)
        bkk_sb = const.tile([128, 1], F32)
        nc.sync.dma_start(out=bkk_sb, in_=bk_d)
        bv_sb = const.tile([VD, 1], F32)
        nc.sync.dma_start(out=bv_sb, in_=bv_d)
        wq_sb = const.tile([128, 4, KD], F16)
        nc.sync.dma_start(out=wq_sb, in_=wq_d.rearrange("a p m -> p a m"))
        wk_sb = const.tile([128, 4, KD], F16)
        nc.sync.dma_start(out=wk_sb, in_=wk_d.rearrange("a p m -> p a m"))
        wv_sb = const.tile([128, 4, VD], F16)
        nc.sync.dma_start(out=wv_sb, in_=wv_d.rearrange("a p m -> p a m"))

        # x^T tiles: c-chunk (128 partitions) x col-group (1024 t-cols),
        # loaded in H_ORDER so attention pairs unblock early.
        xts = [
            [data.tile([128, 1024], F16, name=f"xt{c}_{h}") for h in range(4)]
            for c in range(4)
        ]
        for h in H_ORDER:
            for c in range(4):
                if h == 0:
                    for q2 in range(2):
                        nc.sync.dma_start(
                            out=xts[c][h][:, q2 * 512:(q2 + 1) * 512],
                            in_=xT_d[c * 128:(c + 1) * 128,
                                     q2 * 512:(q2 + 1) * 512],
                        )
                else:
                    nc.sync.dma_start(
                        out=xts[c][h],
                        in_=xT_d[c * 128:(c + 1) * 128,
                                 h * 1024:(h + 1) * 1024],
                    )

        # passthrough: own x rows -> out[:, 0:512] directly HBM->HBM
        for i in range(4):
            nc.sync.dma_start(
                out=out_d[i * 512:(i + 1) * 512, 0:C],
                in_=xq_d[i * 512:(i + 1) * 512, :],
            )

        v_ext = data.tile([128, 32, VD + 1], F16)
        # ones column for the fused softmax denominator: (x*0)+1 via
        # tensor_scalar (memset can't cast)
        nc.vector.tensor_scalar(
            out=v_ext[:, :, VD:VD + 1],
            in0=ident_sb[:, 0:32].unsqueeze(2),
            scalar1=0.0, scalar2=1.0,
            op0=mybir.AluOpType.mult, op1=ADD,
        )

        q_sb = [data.tile([128, 512], F16, name=f"q{k}") for k in range(4)]
        k_sb = [data.tile([128, 512], F16, name=f"k{k}") for k in range(8)]
        vt_sb = [data.tile([128, 512], F16, name=f"vt{k}") for k in range(8)]
        for k in range(8):
            nc.gpsimd.memset(vt_sb[k][VD:128, :], 0.0)

        # ---- projections (q^T, k^T, v^T) + v transposition ----
        with tc.tile_pool(name="ps_proj", bufs=3, space="PSUM") as ps_proj, \
             tc.tile_pool(name="ps_vtr", bufs=2, space="PSUM") as ps_vtr:
            for k in K_ORDER:
                h, off = k // 2, (k % 2) * 512
                psk = ps_proj.tile([128, 512], F32, name=f"psk{k}", tag="ps")
                for c in range(4):
                    nc.tensor.matmul(
                        psk[0:KD, :], wk_sb[:, c, :],
                        xts[c][h][:, off:off + 512],
                        start=(c == 0), stop=(c == 3),
                    )
                for c in range(4):
                    nc.tensor.matmul(
                        psk[64:64 + KD, :], wk_sb[:, c, :],
                        xts[c][h][:, off:off + 512],
                        start=(c == 0), stop=(c == 3),
                        tile_position=(0, 64),
                    )
                nc.vector.tensor_scalar(
                    out=k_sb[k], in0=psk, scalar1=bkk_sb, scalar2=None, op0=ADD)
                psv = ps_proj.tile([VD, 512], F32, name=f"psv{k}", tag="ps")
                for c in range(4):
                    nc.tensor.matmul(
                        psv, wv_sb[:, c, :], xts[c][h][:, off:off + 512],
                        start=(c == 0), stop=(c == 3),
                    )
                nc.vector.tensor_scalar(
                    out=vt_sb[k][0:VD, :], in0=psv, scalar1=bv_sb, scalar2=None,
                    op0=ADD)
                if k < 4:
                    psq = ps_proj.tile([128, 512], F32, name=f"psq{k}", tag="ps")
                    for c in range(4):
                        nc.tensor.matmul(
                            psq[0:KD, :], wq_sb[:, c, :],
                            xts[c][h][:, off:off + 512],
                            start=(c == 0), stop=(c == 3),
                        )
                    for c in range(4):
                        nc.tensor.matmul(
                            psq[64:64 + KD, :], wq_sb[:, c, :],
                            xts[c][h][:, off:off + 512],
                            start=(c == 0), stop=(c == 3),
                            tile_position=(0, 64),
                        )
                    nc.vector.tensor_scalar(
                        out=q_sb[k], in0=psq, scalar1=bqq_sb, scalar2=None,
                        op0=ADD)
                # transpose v^T chunk into natural layout blocks
                pvt = ps_vtr.tile([128, 4, 128], F16, name=f"pvt{k}", tag="pvt")
                for j in range(4):
                    nc.tensor.transpose(
                        pvt[:, j, :], vt_sb[k][:, j * 128:(j + 1) * 128],
                        ident16_sb)
                nc.vector.tensor_copy(
                    v_ext[:, 4 * k:4 * k + 4, 0:VD], pvt[:, :, 0:VD])

        # ---- attention over 4 slot pairs ----
        # Group roles within pair u (nG = 4u+4 groups of 2 tk-blocks):
        #   g == 2u      full : own chunk 2u    -> s0 diag tri mask
        #   g == 2u+1  narrow : own chunk 2u+1  -> s1 diag tri mask (s0 dead)
        #   g == nG-2    full : other chunk 2u  -> s0 parity mask
        #   g == nG-1  narrow : other chunk 2u+1-> s1 parity mask (s0 dead)
        # Two pair-streams are interleaved so the PE always has independent
        # matmul work while the other stream waits on its exp.
        with tc.tile_pool(name="ps_o", bufs=4, space="PSUM") as ps_o, \
             tc.tile_pool(name="ptp", bufs=4) as ptp, \
             tc.tile_pool(name="finp", bufs=2) as finp:

            po = [ps_o.tile([VD + 1, 512], F32, name=f"po{u}", tag="po")
                  for u in range(4)]

            with tc.tile_pool(name="ps_l", bufs=2, space="PSUM") as ps_l:

                def emit_group(u, g):
                    nG = 4 * u + 4
                    own = 4 * u + 4
                    qrhs = q_sb[u]

                    def blk(seq):
                        return seq if seq < own else 16 + (seq - own)

                    narrow = g == 2 * u + 1 or g == nG - 1
                    pl = ps_l.tile([128, 2, 512], F32, name=f"pl{u}_{g}",
                                   tag="pl")
                    for j in range(2):
                        b = blk(2 * g + j)
                        half = j * KD
                        kw = k_sb[b // 4][half:half + KD,
                                          (b % 4) * 128:(b % 4) * 128 + 128]
                        if narrow:
                            nc.tensor.matmul(pl[:, j, 0:CH], kw,
                                             qrhs[half:half + KD, CH:512],
                                             start=True, stop=True)
                        else:
                            nc.tensor.matmul(pl[:, j, :], kw,
                                             qrhs[half:half + KD, :],
                                             start=True, stop=True)
                    if g == 2 * u or g == 2 * u + 1:
                        nc.vector.tensor_add(
                            pl[:, :, 0:CH], pl[:, :, 0:CH], masks_sb[:, 0:2, :])
                    elif g >= nG - 2:
                        nc.vector.tensor_add(
                            pl[:, :, 0:CH], pl[:, :, 0:CH], masks_sb[:, 2:4, :])
                    pt = ptp.tile([128, 2, 512], F16, name="pt", tag="pt")
                    if narrow:
                        nc.scalar.activation(out=pt[:, :, 0:CH],
                                             in_=pl[:, :, 0:CH],
                                             func=EXP, scale=SCALE)
                    else:
                        nc.scalar.activation(out=pt, in_=pl, func=EXP,
                                             scale=SCALE)
                    for j in range(2):
                        b = blk(2 * g + j)
                        nc.tensor.matmul(
                            po[u][:, CH:512] if narrow else po[u],
                            v_ext[:, b, :],
                            pt[:, j, 0:CH] if narrow else pt[:, j, :],
                            start=(g == 0 and j == 0),
                            stop=(g == nG - 1 and j == 1),
                        )

                for g in range(16):
                    for u in (3, 2, 1, 0):
                        if g < 4 * u + 4:
                            emit_group(u, g)

            with tc.tile_pool(name="ps_f", bufs=2, space="PSUM") as ps_f:
                for u in range(4):
                    osb = finp.tile([VD + 1, 512], F32, name="osb", tag="osb")
                    nc.vector.tensor_copy(osb, po[u])
                    for hh in range(4):
                        pf = ps_f.tile([128, VD + 1], F32, name=f"pf{u}_{hh}",
                                       tag="pf")
                        nc.tensor.transpose(
                            pf, osb[:, hh * 128:hh * 128 + 128],
                            ident_sb[0:VD + 1, 0:VD + 1])
                        rc = finp.tile([128, 1], F32, name="rc", tag="rc")
                        nc.vector.reciprocal(rc, pf[:, VD:VD + 1])
                        res = finp.tile([128, VD], F32, name="res", tag="res")
                        nc.vector.tensor_scalar_mul(res, pf[:, 0:VD], rc)
                        r0 = u * 512 + hh * 128
                        nc.sync.dma_start(
                            out=out_d[r0:r0 + 128, C:C + VD], in_=res)

    nc.compile()
    return nc


def _get_nc():
    global _NC_CACHE
    if _NC_CACHE is None:
        _NC_CACHE = _build()
    return _NC_CACHE


def _make_masks(p):
    """Additive mask tiles [128, 4, 256] for parity p (0 keeps, -1e30 kills)."""
    NEG = np.float32(-1.0e30)
    i = np.arange(128)[:, None]
    j = np.arange(CH)[None, :]
    m = np.zeros((128, 4, CH), dtype=np.float32)
    m[:, 0, :] = np.where(j >= i, 0.0, NEG)        # diag tri, tk-block 0
    m[:, 1, :] = np.where(j >= 128 + i, 0.0, NEG)  # diag tri, tk-block 1
    m[:, 2:4, :] = 0.0 if p == 1 else NEG          # other-chunk parity mask
    return m


def kernel(x, Wq, bq, Wk, bk, Wv, bv):
    x = np.asarray(x, dtype=np.float32)
    Wq = np.asarray(Wq, dtype=np.float32)
    Wk = np.asarray(Wk, dtype=np.float32)
    Wv = np.asarray(Wv, dtype=np.float32)
    bq = np.asarray(bq, dtype=np.float32)
    bk = np.asarray(bk, dtype=np.float32)
    bv = np.asarray(bv, dtype=np.float32)

    nc = _get_nc()

    ident = np.eye(128, dtype=np.float32)
    ident16 = np.eye(128, dtype=np.float16)
    wq_h = np.ascontiguousarray(Wq.reshape(4, 128, KD).astype(np.float16))
    wk_h = np.ascontiguousarray(Wk.reshape(4, 128, KD).astype(np.float16))
    wv_h = np.ascontiguousarray(Wv.reshape(4, 128, VD).astype(np.float16))
    bq_h = np.ascontiguousarray(np.tile(bq.reshape(KD, 1), (2, 1)))
    bk_h = np.ascontiguousarray(np.tile(bk.reshape(KD, 1), (2, 1)))
    bv_h = np.ascontiguousarray(bv.reshape(VD, 1))
    masks_p = [_make_masks(0), _make_masks(1)]

    in_maps = []
    for core in range(8):
        n, p = core // 2, core % 2
        perm = [2 * s + p for s in range(8)] + [2 * s + 1 - p for s in range(8)]
        xp = x[n].reshape(16, CH, C)[perm].reshape(T, C)
        in_maps.append({
            "xT": np.ascontiguousarray(xp.T.astype(np.float16)),
            "xq": np.ascontiguousarray(xp[:TQ]),
            "wq": wq_h, "wk": wk_h, "wv": wv_h,
            "bq": bq_h, "bk": bk_h, "bv": bv_h,
            "masks": masks_p[p], "ident": ident, "ident16": ident16,
        })

    global _LAST_IN_MAPS
    _LAST_IN_MAPS = in_maps
    res = run_bass_kernel_spmd(nc, in_maps, core_ids=list(range(8)))

    out = np.empty((N, T, C + VD), dtype=np.float32)
    for core in range(8):
        n, p = core // 2, core % 2
        co = res.results[core]["out"]
        for s in range(8):
            g0 = (2 * s + p) * CH
            out[n, g0:g0 + CH] = co[s * CH:(s + 1) * CH]
    return out


# revision 13
# speedup vs baseline: 1.1713x; 1.1713x over previous
"""Causal attention block (q/k/v proj + causal softmax + concat) on 8 trn2 cores.

Sharding: batch n -> core pair (2n, 2n+1); within a batch the 16 query
chunks of 256 rows are split by parity (core p owns chunks 2s+p, s=0..7).
The host hands each core a row-permuted x (own chunks first, then the
other core's chunks) plus its fp16 transpose, so all 8 cores run one
uniform SPMD program; causal-mask differences are pure input data
(multiplicative 0/1 mask tiles). Attention runs in the transposed
orientation (logits^T tiles [tk=128, tq=512]) over PAIRS of query slots,
with fp16 matmul operands (fp32 PSUM accumulation), and the softmax
denominator is fused into the read matmul via a ones-column appended
to v. Groups whose first-slot half is dead by causality are computed at
half width.
"""

from contextlib import ExitStack

import numpy as np

import concourse.bacc as bacc
import concourse.mybir as mybir
import concourse.tile as tile
from concourse.bass_utils import run_bass_kernel_spmd

F32 = mybir.dt.float32
F16 = mybir.dt.float16
ADD = mybir.AluOpType.add
EXP = mybir.ActivationFunctionType.Exp
IDENT = mybir.ActivationFunctionType.Identity

N, T, C, KD, VD = 4, 4096, 512, 64, 64
CH = 256          # query chunk rows
SLOTS = 8         # own 256-chunks per core
TQ = SLOTS * CH   # 2048 own query rows per core
SCALE = 1.0 / 8.0

_NC_CACHE = None
_LAST_IN_MAPS = None

# DMA / projection processing order: own cols (h 0,1) and other cols (h 2,3)
# interleaved so early attention pairs have both parts available.
H_ORDER = [0, 2, 1, 3]
K_ORDER = [0, 1, 4, 5, 2, 3, 6, 7]


def _build():
    nc = bacc.Bacc("TRN2", target_bir_lowering=False, debug=False)

    xT_d = nc.dram_tensor("xT", [C, T], F16, kind="ExternalInput").ap()
    xq_d = nc.dram_tensor("xq", [TQ, C], F32, kind="ExternalInput").ap()
    wq_d = nc.dram_tensor("wq", [4, 128, KD], F16, kind="ExternalInput").ap()
    wk_d = nc.dram_tensor("wk", [4, 128, KD], F16, kind="ExternalInput").ap()
    wv_d = nc.dram_tensor("wv", [4, 128, VD], F16, kind="ExternalInput").ap()
    bq_d = nc.dram_tensor("bq", [128, 1], F32, kind="ExternalInput").ap()
    bk_d = nc.dram_tensor("bk", [128, 1], F32, kind="ExternalInput").ap()
    bv_d = nc.dram_tensor("bv", [VD, 1], F32, kind="ExternalInput").ap()
    masks_d = nc.dram_tensor("masks", [128, 4, CH], F32, kind="ExternalInput").ap()
    ident_d = nc.dram_tensor("ident", [128, 128], F32, kind="ExternalInput").ap()
    ident16_d = nc.dram_tensor("ident16", [128, 128], F16, kind="ExternalInput").ap()
    out_d = nc.dram_tensor("out", [TQ, C + VD], F32, kind="ExternalOutput").ap()

    with tile.TileContext(nc) as tc, ExitStack() as ctx:
        const = ctx.enter_context(tc.tile_pool(name="const", bufs=1))
        data = ctx.enter_context(tc.tile_pool(name="data", bufs=1))

        ident_sb = const.tile([128, 128], F32)
        nc.sync.dma_start(out=ident_sb, in_=ident_d)
        ident16_sb = const.tile([128, 128], F16)
        nc.sync.dma_start(out=ident16_sb, in_=ident16_d)
        masks_sb = const.tile([128, 4, CH], F32)
        nc.sync.dma_start(out=masks_sb, in_=masks_d)
        bqq_sb = const.tile([128, 1], F32)
        nc.sync.dma_start(out=bqq_sb, in_=bq_d)
        bkk_sb = const.tile([128, 1], F32)
        nc.sync.dma_start(out=bkk_sb, in_=bk_d)
        bv_sb = const.tile([VD, 1], F32)
        nc.sync.dma_start(out=bv_sb, in_=bv_d)
        wq_sb = const.tile([128, 4, KD], F16)
        nc.sync.dma_start(out=wq_sb, in_=wq_d.rearrange("a p m -> p a m"))
        wk_sb = const.tile([128, 4, KD], F16)
        nc.sync.dma_start(out=wk_sb, in_=wk_d.rearrange("a p m -> p a m"))
        wv_sb = const.tile([128, 4, VD], F16)
        nc.sync.dma_start(out=wv_sb, in_=wv_d.rearrange("a p m -> p a m"))

        # x^T tiles: c-chunk (128 partitions) x col-group (1024 t-cols),
        # loaded in H_ORDER so attention pairs unblock early.
        xts = [
            [data.tile([128, 1024], F16, name=f"xt{c}_{h}") for h in range(4)]
            for c in range(4)
        ]
        for h in H_ORDER:
            for c in range(4):
                if h == 0:
                    for q2 in range(2):
                        nc.sync.dma_start(
                            out=xts[c][h][:, q2 * 512:(q2 + 1) * 512],
                            in_=xT_d[c * 128:(c + 1) * 128,
                                     q2 * 512:(q2 + 1) * 512],
                        )
                else:
                    nc.sync.dma_start(
                        out=xts[c][h],
                        in_=xT_d[c * 128:(c + 1) * 128,
                                 h * 1024:(h + 1) * 1024],
                    )

        # passthrough: own x rows -> out[:, 0:512] directly HBM->HBM
        for i in range(4):
            nc.sync.dma_start(
                out=out_d[i * 512:(i + 1) * 512, 0:C],
                in_=xq_d[i * 512:(i + 1) * 512, :],
            )

        v_ext = data.tile([128, 32, VD + 1], F16)
        # ones column for the fused softmax denominator: (x*0)+1 via
        # tensor_scalar (memset can't cast)
        nc.vector.tensor_scalar(
            out=v_ext[:, :, VD:VD + 1],
            in0=ident_sb[:, 0:32].unsqueeze(2),
            scalar1=0.0, scalar2=1.0,
            op0=mybir.AluOpType.mult, op1=ADD,
        )

        q_sb = [data.tile([128, 512], F16, name=f"q{k}") for k in range(4)]
        k_sb = [data.tile([128, 512], F16, name=f"k{k}") for k in range(8)]
        vt_sb = [data.tile([128, 512], F16, name=f"vt{k}") for k in range(8)]
        for k in range(8):
            nc.gpsimd.memset(vt_sb[k][VD:128, :], 0.0)

        # ---- projections (q^T, k^T, v^T) + v transposition ----
        with tc.tile_pool(name="ps_proj", bufs=3, space="PSUM") as ps_proj, \
             tc.tile_pool(name="ps_vtr", bufs=2, space="PSUM") as ps_vtr:
            for k in K_ORDER:
                h, off = k // 2, (k % 2) * 512
                psk = ps_proj.tile([128, 512], F32, name=f"psk{k}", tag="ps")
                for c in range(4):
                    nc.tensor.matmul(
                        psk[0:KD, :], wk_sb[:, c, :],
                        xts[c][h][:, off:off + 512],
                        start=(c == 0), stop=(c == 3),
                    )
                for c in range(4):
                    nc.tensor.matmul(
                        psk[64:64 + KD, :], wk_sb[:, c, :],
                        xts[c][h][:, off:off + 512],
                        start=(c == 0), stop=(c == 3),
                        tile_position=(0, 64),
                    )
                nc.scalar.activation(out=k_sb[k], in_=psk, func=IDENT,
                                     bias=bkk_sb, scale=1.0)
                psv = ps_proj.tile([VD, 512], F32, name=f"psv{k}", tag="ps")
                for c in range(4):
                    nc.tensor.matmul(
                        psv, wv_sb[:, c, :], xts[c][h][:, off:off + 512],
                        start=(c == 0), stop=(c == 3),
                    )
                nc.scalar.activation(out=vt_sb[k][0:VD, :], in_=psv,
                                     func=IDENT, bias=bv_sb, scale=1.0)
                if k < 4:
                    psq = ps_proj.tile([128, 512], F32, name=f"psq{k}", tag="ps")
                    for c in range(4):
                        nc.tensor.matmul(
                            psq[0:KD, :], wq_sb[:, c, :],
                            xts[c][h][:, off:off + 512],
                            start=(c == 0), stop=(c == 3),
                        )
                    for c in range(4):
                        nc.tensor.matmul(
                            psq[64:64 + KD, :], wq_sb[:, c, :],
                            xts[c][h][:, off:off + 512],
                            start=(c == 0), stop=(c == 3),
                            tile_position=(0, 64),
                        )
                    nc.scalar.activation(out=q_sb[k], in_=psq, func=IDENT,
                                         bias=bqq_sb, scale=1.0)
                # transpose v^T chunk into natural layout blocks
                pvt = ps_vtr.tile([128, 4, 128], F16, name=f"pvt{k}", tag="pvt")
                for j in range(4):
                    nc.tensor.transpose(
                        pvt[:, j, :], vt_sb[k][:, j * 128:(j + 1) * 128],
                        ident16_sb)
                nc.scalar.activation(
                    out=v_ext[:, 4 * k:4 * k + 4, 0:VD], in_=pvt[:, :, 0:VD],
                    func=IDENT, scale=1.0)

        # ---- attention over 4 slot pairs ----
        # Group roles within pair u (nG = 4u+4 groups of 2 tk-blocks):
        #   g == 2u      full : own chunk 2u    -> s0 diag tri mask
        #   g == 2u+1  narrow : own chunk 2u+1  -> s1 diag tri mask (s0 dead)
        #   g == nG-2    full : other chunk 2u  -> s0 parity mask
        #   g == nG-1  narrow : other chunk 2u+1-> s1 parity mask (s0 dead)
        # Two pair-streams are interleaved so the PE always has independent
        # matmul work while the other stream waits on its exp.
        with tc.tile_pool(name="ps_l", bufs=2, space="PSUM") as ps_l, \
             tc.tile_pool(name="ps_o", bufs=2, space="PSUM") as ps_o, \
             tc.tile_pool(name="ps_f", bufs=2, space="PSUM") as ps_f, \
             tc.tile_pool(name="ptp", bufs=4) as ptp, \
             tc.tile_pool(name="finp", bufs=2) as finp:

            def emit_group(u, g, po):
                nG = 4 * u + 4
                own = 4 * u + 4
                qrhs = q_sb[u]

                def blk(seq):
                    return seq if seq < own else 16 + (seq - own)

                narrow = g == 2 * u + 1 or g == nG - 1
                pl = ps_l.tile([128, 2, 512], F32, name=f"pl{u}_{g}", tag="pl")
                for j in range(2):
                    b = blk(2 * g + j)
                    half = j * KD
                    kw = k_sb[b // 4][half:half + KD,
                                      (b % 4) * 128:(b % 4) * 128 + 128]
                    if narrow:
                        nc.tensor.matmul(pl[:, j, 0:CH], kw,
                                         qrhs[half:half + KD, CH:512],
                                         start=True, stop=True)
                    else:
                        nc.tensor.matmul(pl[:, j, :], kw,
                                         qrhs[half:half + KD, :],
                                         start=True, stop=True)
                if g == 2 * u or g == 2 * u + 1:
                    nc.vector.tensor_add(
                        pl[:, :, 0:CH], pl[:, :, 0:CH], masks_sb[:, 0:2, :])
                elif g >= nG - 2:
                    nc.vector.tensor_add(
                        pl[:, :, 0:CH], pl[:, :, 0:CH], masks_sb[:, 2:4, :])
                pt = ptp.tile([128, 2, 512], F16, name="pt", tag="pt")
                if narrow:
                    nc.scalar.activation(out=pt[:, :, 0:CH], in_=pl[:, :, 0:CH],
                                         func=EXP, scale=SCALE)
                else:
                    nc.scalar.activation(out=pt, in_=pl, func=EXP, scale=SCALE)
                for j in range(2):
                    b = blk(2 * g + j)
                    nc.tensor.matmul(
                        po[:, CH:512] if narrow else po,
                        v_ext[:, b, :],
                        pt[:, j, 0:CH] if narrow else pt[:, j, :],
                        start=(g == 0 and j == 0),
                        stop=(g == nG - 1 and j == 1),
                    )

            def finalize(u, po):
                osb = finp.tile([VD + 1, 512], F32, name="osb", tag="osb")
                nc.vector.tensor_copy(osb, po)
                for hh in range(4):
                    pf = ps_f.tile([128, VD + 1], F32, name=f"pf{u}_{hh}",
                                   tag="pf")
                    nc.tensor.transpose(
                        pf, osb[:, hh * 128:hh * 128 + 128],
                        ident_sb[0:VD + 1, 0:VD + 1])
                    rc = finp.tile([128, 1], F32, name="rc", tag="rc")
                    nc.vector.reciprocal(rc, pf[:, VD:VD + 1])
                    res = finp.tile([128, VD], F32, name="res", tag="res")
                    nc.vector.tensor_scalar_mul(res, pf[:, 0:VD], rc)
                    r0 = u * 512 + hh * 128
                    nc.sync.dma_start(
                        out=out_d[r0:r0 + 128, C:C + VD], in_=res)

            for ua, ub in [(0, 1), (2, 3)]:
                po_a = ps_o.tile([VD + 1, 512], F32, name=f"po{ua}", tag="po")
                po_b = ps_o.tile([VD + 1, 512], F32, name=f"po{ub}", tag="po")
                na, nb = 4 * ua + 4, 4 * ub + 4
                for g in range(max(na, nb)):
                    if g < nb:
                        emit_group(ub, g, po_b)
                    if g < na:
                        emit_group(ua, g, po_a)
                finalize(ua, po_a)
                finalize(ub, po_b)

    nc.compile()
    return nc


def _get_nc():
    global _NC_CACHE
    if _NC_CACHE is None:
        _NC_CACHE = _build()
    return _NC_CACHE


def _make_masks(p):
    """Additive mask tiles [128, 4, 256] for parity p (0 keeps, -1e30 kills)."""
    NEG = np.float32(-1.0e30)
    i = np.arange(128)[:, None]
    j = np.arange(CH)[None, :]
    m = np.zeros((128, 4, CH), dtype=np.float32)
    m[:, 0, :] = np.where(j >= i, 0.0, NEG)        # diag tri, tk-block 0
    m[:, 1, :] = np.where(j >= 128 + i, 0.0, NEG)  # diag tri, tk-block 1
    m[:, 2:4, :] = 0.0 if p == 1 else NEG          # other-chunk parity mask
    return m


def kernel(x, Wq, bq, Wk, bk, Wv, bv):
    x = np.asarray(x, dtype=np.float32)
    Wq = np.asarray(Wq, dtype=np.float32)
    Wk = np.asarray(Wk, dtype=np.float32)
    Wv = np.asarray(Wv, dtype=np.float32)
    bq = np.asarray(bq, dtype=np.float32)
    bk = np.asarray(bk, dtype=np.float32)
    bv = np.asarray(bv, dtype=np.float32)

    nc = _get_nc()

    ident = np.eye(128, dtype=np.float32)
    ident16 = np.eye(128, dtype=np.float16)
    wq_h = np.ascontiguousarray(Wq.reshape(4, 128, KD).astype(np.float16))
    wk_h = np.ascontiguousarray(Wk.reshape(4, 128, KD).astype(np.float16))
    wv_h = np.ascontiguousarray(Wv.reshape(4, 128, VD).astype(np.float16))
    bq_h = np.ascontiguousarray(np.tile(bq.reshape(KD, 1), (2, 1)))
    bk_h = np.ascontiguousarray(np.tile(bk.reshape(KD, 1), (2, 1)))
    bv_h = np.ascontiguousarray(bv.reshape(VD, 1))
    masks_p = [_make_masks(0), _make_masks(1)]

    in_maps = []
    for core in range(8):
        n, p = core // 2, core % 2
        perm = [2 * s + p for s in range(8)] + [2 * s + 1 - p for s in range(8)]
        xp = x[n].reshape(16, CH, C)[perm].reshape(T, C)
        in_maps.append({
            "xT": np.ascontiguousarray(xp.T.astype(np.float16)),
            "xq": np.ascontiguousarray(xp[:TQ]),
            "wq": wq_h, "wk": wk_h, "wv": wv_h,
            "bq": bq_h, "bk": bk_h, "bv": bv_h,
            "masks": masks_p[p], "ident": ident, "ident16": ident16,
        })

    global _LAST_IN_MAPS
    _LAST_IN_MAPS = in_maps
    res = run_bass_kernel_spmd(nc, in_maps, core_ids=list(range(8)))

    out = np.empty((N, T, C + VD), dtype=np.float32)
    for core in range(8):
        n, p = core // 2, core % 2
        co = res.results[core]["out"]
        for s in range(8):
            g0 = (2 * s + p) * CH
            out[n, g0:g0 + CH] = co[s * CH:(s + 1) * CH]
    return out


# revision 14
# speedup vs baseline: 1.2219x; 1.0432x over previous
"""Causal attention block (q/k/v proj + causal softmax + concat) on 8 trn2 cores.

Sharding: batch n -> core pair (2n, 2n+1); within a batch the 16 query
chunks of 256 rows are split by parity (core p owns chunks 2s+p, s=0..7).
The host hands each core a row-permuted x (own chunks first, then the
other core's chunks) plus its fp16 transpose, so all 8 cores run one
uniform SPMD program; causal-mask differences are pure input data
(multiplicative 0/1 mask tiles). Attention runs in the transposed
orientation (logits^T tiles [tk=128, tq=512]) over PAIRS of query slots,
with fp16 matmul operands (fp32 PSUM accumulation), and the softmax
denominator is fused into the read matmul via a ones-column appended
to v. Groups whose first-slot half is dead by causality are computed at
half width.
"""

from contextlib import ExitStack

import numpy as np

import concourse.bacc as bacc
import concourse.mybir as mybir
import concourse.tile as tile
from concourse.bass_utils import run_bass_kernel_spmd

F32 = mybir.dt.float32
F16 = mybir.dt.float16
ADD = mybir.AluOpType.add
EXP = mybir.ActivationFunctionType.Exp

N, T, C, KD, VD = 4, 4096, 512, 64, 64
CH = 256          # query chunk rows
SLOTS = 8         # own 256-chunks per core
TQ = SLOTS * CH   # 2048 own query rows per core
SCALE = 1.0 / 8.0

_NC_CACHE = None
_LAST_IN_MAPS = None

# DMA / projection processing order: own cols (h 0,1) and other cols (h 2,3)
# interleaved so early attention pairs have both parts available.
H_ORDER = [0, 2, 1, 3]
K_ORDER = [0, 1, 4, 5, 2, 3, 6, 7]


def _build():
    nc = bacc.Bacc("TRN2", target_bir_lowering=False, debug=False)

    xT_d = nc.dram_tensor("xT", [C, T], F16, kind="ExternalInput").ap()
    xq_d = nc.dram_tensor("xq", [TQ, C], F32, kind="ExternalInput").ap()
    wq_d = nc.dram_tensor("wq", [4, 128, KD], F16, kind="ExternalInput").ap()
    wk_d = nc.dram_tensor("wk", [4, 128, KD], F16, kind="ExternalInput").ap()
    wv_d = nc.dram_tensor("wv", [4, 128, VD], F16, kind="ExternalInput").ap()
    bq_d = nc.dram_tensor("bq", [128, 1], F32, kind="ExternalInput").ap()
    bk_d = nc.dram_tensor("bk", [128, 1], F32, kind="ExternalInput").ap()
    bv_d = nc.dram_tensor("bv", [VD, 1], F32, kind="ExternalInput").ap()
    masks_d = nc.dram_tensor("masks", [128, 4, CH], F16, kind="ExternalInput").ap()
    ident_d = nc.dram_tensor("ident", [128, 128], F32, kind="ExternalInput").ap()
    ident16_d = nc.dram_tensor("ident16", [128, 128], F16, kind="ExternalInput").ap()
    out_d = nc.dram_tensor("out", [TQ, C + VD], F32, kind="ExternalOutput").ap()

    with tile.TileContext(nc) as tc, ExitStack() as ctx:
        const = ctx.enter_context(tc.tile_pool(name="const", bufs=1))
        data = ctx.enter_context(tc.tile_pool(name="data", bufs=1))

        ident_sb = const.tile([128, 128], F32)
        nc.sync.dma_start(out=ident_sb, in_=ident_d)
        ident16_sb = const.tile([128, 128], F16)
        nc.sync.dma_start(out=ident16_sb, in_=ident16_d)
        masks_sb = const.tile([128, 4, CH], F16)
        nc.sync.dma_start(out=masks_sb, in_=masks_d)
        bqq_sb = const.tile([128, 1], F32)
        nc.sync.dma_start(out=bqq_sb, in_=bq_d)
        bkk_sb = const.tile([128, 1], F32)
        nc.sync.dma_start(out=bkk_sb, in_=bk_d)
        bv_sb = const.tile([VD, 1], F32)
        nc.sync.dma_start(out=bv_sb, in_=bv_d)
        wq_sb = const.tile([128, 4, KD], F16)
        nc.sync.dma_start(out=wq_sb, in_=wq_d.rearrange("a p m -> p a m"))
        wk_sb = const.tile([128, 4, KD], F16)
        nc.sync.dma_start(out=wk_sb, in_=wk_d.rearrange("a p m -> p a m"))
        wv_sb = const.tile([128, 4, VD], F16)
        nc.sync.dma_start(out=wv_sb, in_=wv_d.rearrange("a p m -> p a m"))

        # x^T tiles: c-chunk (128 partitions) x col-group (1024 t-cols),
        # loaded in H_ORDER so attention pairs unblock early.
        xts = [
            [data.tile([128, 1024], F16, name=f"xt{c}_{h}") for h in range(4)]
            for c in range(4)
        ]
        for h in H_ORDER:
            for c in range(4):
                if h == 0:
                    for q2 in range(2):
                        nc.sync.dma_start(
                            out=xts[c][h][:, q2 * 512:(q2 + 1) * 512],
                            in_=xT_d[c * 128:(c + 1) * 128,
                                     q2 * 512:(q2 + 1) * 512],
                        )
                else:
                    nc.sync.dma_start(
                        out=xts[c][h],
                        in_=xT_d[c * 128:(c + 1) * 128,
                                 h * 1024:(h + 1) * 1024],
                    )

        # passthrough: own x rows -> out[:, 0:512] directly HBM->HBM
        for i in range(4):
            nc.sync.dma_start(
                out=out_d[i * 512:(i + 1) * 512, 0:C],
                in_=xq_d[i * 512:(i + 1) * 512, :],
            )

        v_ext = data.tile([128, 32, VD + 1], F16)
        # ones column for the fused softmax denominator: (x*0)+1 via
        # tensor_scalar (memset can't cast)
        nc.vector.tensor_scalar(
            out=v_ext[:, :, VD:VD + 1],
            in0=ident_sb[:, 0:32].unsqueeze(2),
            scalar1=0.0, scalar2=1.0,
            op0=mybir.AluOpType.mult, op1=ADD,
        )

        q_sb = [data.tile([128, 512], F16, name=f"q{k}") for k in range(4)]
        k_sb = [data.tile([128, 512], F16, name=f"k{k}") for k in range(8)]
        vt_sb = [data.tile([128, 512], F16, name=f"vt{k}") for k in range(8)]
        for k in range(8):
            nc.gpsimd.memset(vt_sb[k][VD:128, :], 0.0)

        # ---- projections (q^T, k^T, v^T) + v transposition ----
        with tc.tile_pool(name="ps_proj", bufs=3, space="PSUM") as ps_proj, \
             tc.tile_pool(name="ps_vtr", bufs=2, space="PSUM") as ps_vtr:
            for k in K_ORDER:
                h, off = k // 2, (k % 2) * 512
                psk = ps_proj.tile([128, 512], F32, name=f"psk{k}", tag="ps")
                for c in range(4):
                    nc.tensor.matmul(
                        psk[0:KD, :], wk_sb[:, c, :],
                        xts[c][h][:, off:off + 512],
                        start=(c == 0), stop=(c == 3),
                    )
                for c in range(4):
                    nc.tensor.matmul(
                        psk[64:64 + KD, :], wk_sb[:, c, :],
                        xts[c][h][:, off:off + 512],
                        start=(c == 0), stop=(c == 3),
                        tile_position=(0, 64),
                    )
                nc.vector.tensor_scalar(
                    out=k_sb[k], in0=psk, scalar1=bkk_sb, scalar2=None, op0=ADD)
                psv = ps_proj.tile([VD, 512], F32, name=f"psv{k}", tag="ps")
                for c in range(4):
                    nc.tensor.matmul(
                        psv, wv_sb[:, c, :], xts[c][h][:, off:off + 512],
                        start=(c == 0), stop=(c == 3),
                    )
                nc.vector.tensor_scalar(
                    out=vt_sb[k][0:VD, :], in0=psv, scalar1=bv_sb, scalar2=None,
                    op0=ADD)
                if k < 4:
                    psq = ps_proj.tile([128, 512], F32, name=f"psq{k}", tag="ps")
                    for c in range(4):
                        nc.tensor.matmul(
                            psq[0:KD, :], wq_sb[:, c, :],
                            xts[c][h][:, off:off + 512],
                            start=(c == 0), stop=(c == 3),
                        )
                    for c in range(4):
                        nc.tensor.matmul(
                            psq[64:64 + KD, :], wq_sb[:, c, :],
                            xts[c][h][:, off:off + 512],
                            start=(c == 0), stop=(c == 3),
                            tile_position=(0, 64),
                        )
                    nc.vector.tensor_scalar(
                        out=q_sb[k], in0=psq, scalar1=bqq_sb, scalar2=None,
                        op0=ADD)
                # transpose v^T chunk into natural layout blocks
                pvt = ps_vtr.tile([128, 4, 128], F16, name=f"pvt{k}", tag="pvt")
                for j in range(4):
                    nc.tensor.transpose(
                        pvt[:, j, :], vt_sb[k][:, j * 128:(j + 1) * 128],
                        ident16_sb)
                nc.vector.tensor_copy(
                    v_ext[:, 4 * k:4 * k + 4, 0:VD], pvt[:, :, 0:VD])

        # ---- attention over 4 slot pairs ----
        # Group roles within pair u (nG = 4u+4 groups of 2 tk-blocks):
        #   g == 2u      full : own chunk 2u    -> s0 diag tri mask
        #   g == 2u+1  narrow : own chunk 2u+1  -> s1 diag tri mask (s0 dead)
        #   g == nG-2    full : other chunk 2u  -> s0 parity mask
        #   g == nG-1  narrow : other chunk 2u+1-> s1 parity mask (s0 dead)
        # Two pair-streams are interleaved so the PE always has independent
        # matmul work while the other stream waits on its exp.
        with tc.tile_pool(name="ps_l", bufs=3, space="PSUM") as ps_l, \
             tc.tile_pool(name="ps_o", bufs=2, space="PSUM") as ps_o, \
             tc.tile_pool(name="ptp", bufs=4) as ptp, \
             tc.tile_pool(name="finp", bufs=2) as finp:

            def emit_group(u, g, po):
                nG = 4 * u + 4
                own = 4 * u + 4
                qrhs = q_sb[u]

                def blk(seq):
                    return seq if seq < own else 16 + (seq - own)

                narrow = g == 2 * u + 1 or g == nG - 1
                pl = ps_l.tile([128, 2, 512], F32, name=f"pl{u}_{g}", tag="pl")
                for j in range(2):
                    b = blk(2 * g + j)
                    half = j * KD
                    kw = k_sb[b // 4][half:half + KD,
                                      (b % 4) * 128:(b % 4) * 128 + 128]
                    if narrow:
                        nc.tensor.matmul(pl[:, j, 0:CH], kw,
                                         qrhs[half:half + KD, CH:512],
                                         start=True, stop=True)
                    else:
                        nc.tensor.matmul(pl[:, j, :], kw,
                                         qrhs[half:half + KD, :],
                                         start=True, stop=True)
                pt = ptp.tile([128, 2, 512], F16, name="pt", tag="pt")
                if narrow:
                    nc.scalar.activation(out=pt[:, :, 0:CH], in_=pl[:, :, 0:CH],
                                         func=EXP, scale=SCALE)
                else:
                    nc.scalar.activation(out=pt, in_=pl, func=EXP, scale=SCALE)
                if g == 2 * u or g == 2 * u + 1:
                    nc.vector.tensor_mul(
                        pt[:, :, 0:CH], pt[:, :, 0:CH], masks_sb[:, 0:2, :])
                elif g >= nG - 2:
                    nc.vector.tensor_mul(
                        pt[:, :, 0:CH], pt[:, :, 0:CH], masks_sb[:, 2:4, :])
                for j in range(2):
                    b = blk(2 * g + j)
                    nc.tensor.matmul(
                        po[:, CH:512] if narrow else po,
                        v_ext[:, b, :],
                        pt[:, j, 0:CH] if narrow else pt[:, j, :],
                        start=(g == 0 and j == 0),
                        stop=(g == nG - 1 and j == 1),
                    )

            def finalize(u, po):
                osb = finp.tile([VD + 1, 512], F32, name="osb", tag="osb")
                nc.vector.tensor_copy(osb, po)
                for hh in range(4):
                    pf = ps_l.tile([128, VD + 1], F32, name=f"pf{u}_{hh}",
                                   tag="pl")
                    nc.tensor.transpose(
                        pf, osb[:, hh * 128:hh * 128 + 128],
                        ident_sb[0:VD + 1, 0:VD + 1])
                    rc = finp.tile([128, 1], F32, name="rc", tag="rc")
                    nc.vector.reciprocal(rc, pf[:, VD:VD + 1])
                    res = finp.tile([128, VD], F32, name="res", tag="res")
                    nc.vector.tensor_scalar_mul(res, pf[:, 0:VD], rc)
                    r0 = u * 512 + hh * 128
                    nc.sync.dma_start(
                        out=out_d[r0:r0 + 128, C:C + VD], in_=res)

            for ua, ub in [(0, 1), (2, 3)]:
                po_a = ps_o.tile([VD + 1, 512], F32, name=f"po{ua}", tag="po")
                po_b = ps_o.tile([VD + 1, 512], F32, name=f"po{ub}", tag="po")
                na, nb = 4 * ua + 4, 4 * ub + 4
                for g in range(max(na, nb)):
                    if g < nb:
                        emit_group(ub, g, po_b)
                    if g < na:
                        emit_group(ua, g, po_a)
                finalize(ua, po_a)
                finalize(ub, po_b)

    nc.compile()
    return nc


def _get_nc():
    global _NC_CACHE
    if _NC_CACHE is None:
        _NC_CACHE = _build()
    return _NC_CACHE


def _make_masks(p):
    """Multiplicative 0/1 fp16 mask tiles [128, 4, 256] for parity p."""
    i = np.arange(128)[:, None]
    j = np.arange(CH)[None, :]
    m = np.zeros((128, 4, CH), dtype=np.float16)
    m[:, 0, :] = (j >= i)             # diag tri, tk-block 0
    m[:, 1, :] = (j >= 128 + i)       # diag tri, tk-block 1
    m[:, 2:4, :] = 1.0 if p == 1 else 0.0   # other-chunk parity mask
    return m


def kernel(x, Wq, bq, Wk, bk, Wv, bv):
    x = np.asarray(x, dtype=np.float32)
    Wq = np.asarray(Wq, dtype=np.float32)
    Wk = np.asarray(Wk, dtype=np.float32)
    Wv = np.asarray(Wv, dtype=np.float32)
    bq = np.asarray(bq, dtype=np.float32)
    bk = np.asarray(bk, dtype=np.float32)
    bv = np.asarray(bv, dtype=np.float32)

    nc = _get_nc()

    ident = np.eye(128, dtype=np.float32)
    ident16 = np.eye(128, dtype=np.float16)
    wq_h = np.ascontiguousarray(Wq.reshape(4, 128, KD).astype(np.float16))
    wk_h = np.ascontiguousarray(Wk.reshape(4, 128, KD).astype(np.float16))
    wv_h = np.ascontiguousarray(Wv.reshape(4, 128, VD).astype(np.float16))
    bq_h = np.ascontiguousarray(np.tile(bq.reshape(KD, 1), (2, 1)))
    bk_h = np.ascontiguousarray(np.tile(bk.reshape(KD, 1), (2, 1)))
    bv_h = np.ascontiguousarray(bv.reshape(VD, 1))
    masks_p = [_make_masks(0), _make_masks(1)]

    in_maps = []
    for core in range(8):
        n, p = core // 2, core % 2
        perm = [2 * s + p for s in range(8)] + [2 * s + 1 - p for s in range(8)]
        xp = x[n].reshape(16, CH, C)[perm].reshape(T, C)
        in_maps.append({
            "xT": np.ascontiguousarray(xp.T.astype(np.float16)),
            "xq": np.ascontiguousarray(xp[:TQ]),
            "wq": wq_h, "wk": wk_h, "wv": wv_h,
            "bq": bq_h, "bk": bk_h, "bv": bv_h,
            "masks": masks_p[p], "ident": ident, "ident16": ident16,
        })

    global _LAST_IN_MAPS
    _LAST_IN_MAPS = in_maps
    res = run_bass_kernel_spmd(nc, in_maps, core_ids=list(range(8)))

    out = np.empty((N, T, C + VD), dtype=np.float32)
    for core in range(8):
        n, p = core // 2, core % 2
        co = res.results[core]["out"]
        for s in range(8):
            g0 = (2 * s + p) * CH
            out[n, g0:g0 + CH] = co[s * CH:(s + 1) * CH]
    return out
